# revision 14
# baseline (speedup 1.0000x reference)
"""DenseGIN (3-layer, dense adjacency) Trainium2 Bass kernel, 8-core SPMD.

Problem: x:(4,4096,2,32) f32, adj:(4,4096,4096) f32 binary, mask:(4,4096) bool.
Per layer l: agg = (adj+I) @ xf ; h = relu(agg@Wa+ba); h = BN(h); h = h@Wb+bb;
x = mask*h ; between layers an outer BN is applied at masked nodes.

Sharding: 8 cores = (batch b, node-half). Core (2b+h) owns output nodes
[h*2048,(h+1)*2048) of batch b.

v3 design (rel-err gate is 2e-2; measured margins from exact host sim):
- Adjacency cast to fp8 e4m3 on host (exact: entries are 0/1), RESIDENT in
  SBUF (8 MiB) in pair-interleaved tiles adjp[t] = [128, 2, 2048] covering
  input node tiles (2t, 2t+1).
- Layers 0/1 aggregate with fp8 DoubleRow matmuls (2 k-tiles per pass, 2x
  bf16 MACs/instruction); layer 2 uses bf16 stationary x against the same
  fp8 adjacency (mixed-dtype, measured OK). Host sim: rel err 8.4e-3.
- Every layer runs as two half-layers of 2 output chunks (512 nodes each):
  aggregation is tile-major (each pair streams into both live chunk PSUMs),
  and each chunk's MLP/epilogue/AllGather overlaps the next wave's
  aggregation. PSUM stays within 8 banks.
- Epilogue: xn PSUM is pre-initialized with the folded bias D via a
  ones-stationary matmul, so the epilogue is a single DVE multiply with a
  host-built per-chunk mask block (casts to fp8/bf16/f32 on write).
- Engine queues are specialized so no queue blocks a collective launch:
  sync = 8 adjacency + const loads + agin/out ships; scalar(ACT) =
  8 adjacency + relus + AG readbacks; gpsimd = memsets/x0/mask/maskblk +
  the 8 AllGathers only; vector(DVE) = agg-copy casts + mask multiplies.
- Output written p-major [128, 16*64] and unshuffled on host.
"""

import sys

if "/opt/trn_rl_repo" not in sys.path:  # PYTHONPATH normally provides it
    sys.path.insert(0, "/opt/trn_rl_repo")

import contextlib
import ctypes
import types

import numpy as np
import ml_dtypes

import concourse.bass as bass
import concourse.tile as tile
from concourse import mybir
from concourse.vector_clock import ScopedClock
import concourse.bass_utils as bass_utils
from concourse.bass_utils import run_bass_kernel_spmd

# ---------------------------------------------------------------------------
# Workaround: the walrus build in this container rejects instructions with
# more than one sem wait ("Too many sync wait commands").  Tile's final drain
# attaches one wait per live semaphore; split them across chained SP drains.
_MAX_WAITS_PER_INST = 1


def _patched_drain_and_barrier(self, tick_clock, wait_clock):
    nc = self.nc
    drain_inst = nc.sync.drain()
    wait_clock.add_sem_waits(drain_inst.ins, ScopedClock({None: tick_clock.global_clock}))
    si = drain_inst.ins.sync_info
    waits = list(si.on_wait or [])
    if len(waits) > _MAX_WAITS_PER_INST:
        si.on_wait = waits[:_MAX_WAITS_PER_INST]
        rest = waits[_MAX_WAITS_PER_INST:]
        for i in range(0, len(rest), _MAX_WAITS_PER_INST):
            extra = nc.sync.drain()
            extra.ins.sync_info = mybir.SyncInfo(
                on_wait=rest[i : i + _MAX_WAITS_PER_INST], on_update=[]
            )
    nc.all_engine_barrier()
    assert self.sems is not None
    popped = nc._tile_sem_poison_stack.pop()
    assert popped is self._sem_poison
    nc.clear_and_free_semaphores(list(self.sems.allocated().values()))
    nc.all_engine_barrier()


tile.TileContext._drain_and_barrier = _patched_drain_and_barrier


def _legalize_sync_waits(nc, max_waits=_MAX_WAITS_PER_INST):
    """Split instructions carrying more than ``max_waits`` sem waits.

    Engine sequencers process their instruction stream in order and execute
    sem waits before dispatch, so hoisting excess waits onto NoOps placed
    just before the instruction (same engine) is semantics-preserving.
    """
    n_split = 0
    for fn in nc.m.functions:
        for blk in fn.blocks:
            insts = blk.instructions
            i = 0
            while i < len(insts):
                inst = insts[i]
                si = inst.sync_info
                waits = list(si.on_wait) if si and si.on_wait else []
                if len(waits) > max_waits:
                    extra, keep = waits[:-max_waits], waits[-max_waits:]
                    si.on_wait = keep
                    pos = i
                    for j in range(0, len(extra), max_waits):
                        nop = mybir.InstNoOp(name=f"I-lsw{n_split}-{j}", ins=[], outs=[])
                        nop.engine = inst.engine
                        nop.sync_info = mybir.SyncInfo(
                            on_wait=extra[j : j + max_waits], on_update=[]
                        )
                        insts.insert(pos, nop)
                        pos += 1
                        i += 1
                    n_split += 1
                i += 1
    return n_split


# ---------------------------------------------------------------------------
# NTFF profiling hook (antenv.axon_hooks is absent in this image).  Only used
# when run() is called with trace=True; registering it is harmless otherwise.
def _ntff_profile_via_ctypes(so_path):
    try:
        lib = ctypes.CDLL(so_path)
    except OSError:
        return None
    if not hasattr(lib, "axon_start_nrt_profile"):
        return None
    lib.axon_start_nrt_profile.argtypes = [ctypes.POINTER(ctypes.c_int64), ctypes.c_size_t]
    lib.axon_start_nrt_profile.restype = ctypes.c_int64
    lib.axon_stop_nrt_profile.argtypes = [ctypes.c_char_p]
    lib.axon_stop_nrt_profile.restype = ctypes.c_int64

    @contextlib.contextmanager
    def _hook(output_dir, device_ids):
        import jax

        jax.devices()
        if device_ids:
            ids = (ctypes.c_int64 * len(device_ids))(*device_ids)
            rc = lib.axon_start_nrt_profile(ids, len(device_ids))
        else:
            rc = lib.axon_start_nrt_profile(None, 0)
        if rc != 0:
            raise RuntimeError(f"axon_start_nrt_profile rc={rc}")
        try:
            yield
        finally:
            n = lib.axon_stop_nrt_profile(str(output_dir).encode())
            print(f"ntff profile: {n} file(s) written to {output_dir}", file=sys.stderr)

    return _hook


if "antenv.axon_hooks" not in sys.modules:
    _hooks_mod = types.ModuleType("antenv.axon_hooks")
    _hook_inst = _ntff_profile_via_ctypes("/opt/axon/libaxon_pjrt.so")
    _hooks_mod.get_axon_ntff_profile_hook = lambda: _hook_inst
    sys.modules["antenv.axon_hooks"] = _hooks_mod
bass_utils.upload_artifacts = lambda tmpdir: f"local:{tmpdir}"

# ---------------------------------------------------------------------------
B, N, K, C_IN, H, C_OUT = 4, 4096, 2, 32, 64, 32
BN_EPS = 1e-5
N_CORES = 8
HALF = N // 2          # 2048 output nodes per core
NPAIR = 16             # 16 pairs of 128-node input tiles
KC_IN = [K * C_IN, K * H, K * H]     # flat input channels per layer: 64,128,128
KC_OUT = [K * H, K * H, K * C_OUT]   # flat output channels per layer: 128,128,64

BF16 = ml_dtypes.bfloat16
FP8 = ml_dtypes.float8_e4m3

_PROGRAM_CACHE = {}


def _build_program():
    """Build the SPMD Bass/Tile program (identical on all 8 cores)."""
    nc = bass.Bass("TRN2", target_bir_lowering=False, debug=False, num_devices=N_CORES)
    dt = mybir.dt
    DR = mybir.MatmulPerfMode.DoubleRow
    RG = [[0, 1], [2, 3], [4, 5], [6, 7]]

    adjp_d = nc.dram_tensor("adjp", [NPAIR * 128, 2 * HALF], dt.float8e4, kind="ExternalInput").ap()
    x0q_d = nc.dram_tensor("x0q", [128, 32 * KC_IN[0]], dt.float8e4, kind="ExternalInput").ap()
    mblk01_d = nc.dram_tensor("mblk01", [128, 16 * KC_OUT[0]], dt.bfloat16, kind="ExternalInput").ap()
    mblk2_d = nc.dram_tensor("mblk2", [128, 16 * KC_OUT[2]], dt.bfloat16, kind="ExternalInput").ap()
    wa0_d = nc.dram_tensor("wa0", [64, 128], dt.bfloat16, kind="ExternalInput").ap()
    wa1a_d = nc.dram_tensor("wa1a", [64, 128], dt.bfloat16, kind="ExternalInput").ap()
    wa1b_d = nc.dram_tensor("wa1b", [64, 128], dt.bfloat16, kind="ExternalInput").ap()
    wa2_d = nc.dram_tensor("wa2", [128, 128], dt.bfloat16, kind="ExternalInput").ap()
    wb_d = [
        nc.dram_tensor(f"wb{l}", [128, KC_OUT[l]], dt.bfloat16, kind="ExternalInput").ap()
        for l in range(3)
    ]
    ba_d = [
        nc.dram_tensor(f"ba{l}", [128, 1], dt.float32, kind="ExternalInput").ap()
        for l in range(3)
    ]
    d4v_d = [
        nc.dram_tensor(f"d4v{l}", [128, 4 * KC_OUT[l]], dt.bfloat16, kind="ExternalInput").ap()
        for l in range(3)
    ]
    out_d = nc.dram_tensor("out", [128, 16 * KC_OUT[2]], dt.float32, kind="ExternalOutput").ap()

    with tile.TileContext(nc) as tc:
        with (
            tc.tile_pool(name="const", bufs=1) as cpool,
            tc.tile_pool(name="xio", bufs=1) as xpool,
            tc.tile_pool(name="work", bufs=3) as wpool,
            tc.tile_pool(name="ps_agg", bufs=1, space="PSUM") as ps_agg,
            tc.tile_pool(name="ps_mlp", bufs=2, space="PSUM") as ps_mlp,
            tc.tile_pool(name="dram", bufs=1, space="DRAM") as dpool,
        ):
            # ---- warmup operands first on gpsimd so the PE can start
            # ramping the HAM clock immediately ----
            wu_lhs = cpool.tile([128, 128], dt.bfloat16, tag="wu_lhs")
            wu_rhs = cpool.tile([128, 512], dt.bfloat16, tag="wu_rhs")
            ones_sb = cpool.tile([128, 128], dt.bfloat16, tag="ones")
            nc.gpsimd.memset(wu_lhs[:], 0.0)
            nc.gpsimd.memset(wu_rhs[:], 0.0)
            nc.gpsimd.memset(ones_sb[:], 1.0)
            wu_ps = ps_mlp.tile([128, 512], dt.float32, tag="h1")
            for _ in range(8):
                nc.tensor.matmul(wu_ps[:], wu_lhs[:], wu_rhs[:], start=True, stop=True)

            # pre-trigger the Relu ACT table load off the critical path
            wu_act = cpool.tile([128, 16], dt.float32, tag="wu_act")
            nc.scalar.activation(
                wu_act[:], wu_lhs[:, 0:16], mybir.ActivationFunctionType.Relu
            )

            # ---- input DMAs; adjacency split over the sync+scalar queues ----
            x0_sb = xpool.tile([128, 32, KC_IN[0]], dt.float8e4, tag="x0")
            nc.gpsimd.dma_start(x0_sb[:, :, :], x0q_d[:, :])
            adjp_sb = [
                cpool.tile([128, 2, HALF], dt.float8e4, tag=f"adjp{t}", name=f"adjp_{t}")
                for t in range(NPAIR)
            ]
            for t in range(NPAIR):
                src = adjp_d[t * 128 : (t + 1) * 128, :]
                if t % 2 == 0:
                    nc.sync.dma_start(adjp_sb[t][:, :, :], src)
                else:
                    nc.scalar.dma_start(adjp_sb[t][:, :, :], src)
            mblk01_sb = cpool.tile([128, 16 * KC_OUT[0]], dt.bfloat16, tag="mblk01")
            nc.gpsimd.dma_start(mblk01_sb[:], mblk01_d[:])
            mblk2_sb = cpool.tile([128, 16 * KC_OUT[2]], dt.bfloat16, tag="mblk2")
            nc.gpsimd.dma_start(mblk2_sb[:], mblk2_d[:])

            # small constants on the gpsimd queue (idle after x0/mblk)
            wa0_sb = cpool.tile([64, 128], dt.bfloat16, tag="wa0")
            nc.gpsimd.dma_start(wa0_sb[:], wa0_d[:])
            wa1a_sb = cpool.tile([64, 128], dt.bfloat16, tag="wa1a")
            nc.gpsimd.dma_start(wa1a_sb[:], wa1a_d[:])
            wa1b_sb = cpool.tile([64, 128], dt.bfloat16, tag="wa1b")
            nc.gpsimd.dma_start(wa1b_sb[:], wa1b_d[:])
            wa2_sb = cpool.tile([128, 128], dt.bfloat16, tag="wa2")
            nc.gpsimd.dma_start(wa2_sb[:], wa2_d[:])
            wb_sb, ba_sb, d4v_sb = [], [], []
            for l in range(3):
                wb = cpool.tile([128, KC_OUT[l]], dt.bfloat16, tag=f"wb{l}", name=f"wb{l}")
                nc.gpsimd.dma_start(wb[:], wb_d[l][:])
                wb_sb.append(wb)
                ba = cpool.tile([128, 1], dt.float32, tag=f"ba{l}", name=f"ba{l}")
                nc.gpsimd.dma_start(ba[:], ba_d[l][:])
                ba_sb.append(ba)
                d4v = cpool.tile([128, 4 * KC_OUT[l]], dt.bfloat16, tag=f"d4v{l}", name=f"d4v{l}")
                nc.gpsimd.dma_start(d4v[:], d4v_d[l][:])
                d4v_sb.append(d4v)

            # AllGather bounce buffers + gathered-x tiles, one AG per
            # half-layer (2 chunks). agin/agout are p-major [128, 2*4*kco];
            # agout rows [rk*128,+128) are rank rk's half. xr[rk][c][p,u,ch]
            # = x for global node tile rk*16 + c*4 + u, row u*128+p.
            def make_ag(l, kco, dtt):
                agin = [
                    dpool.tile([128, 8 * kco], dtt, tag=f"agin{l}_{h}", name=f"agin{l}_{h}")
                    for h in range(2)
                ]
                agout = [
                    dpool.tile([256, 8 * kco], dtt, tag=f"agout{l}_{h}", name=f"agout{l}_{h}")
                    for h in range(2)
                ]
                xr = [
                    [
                        xpool.tile([128, 4, kco], dtt, tag=f"xr{l}_{rk}_{c}", name=f"xr{l}_{rk}_{c}")
                        for c in range(4)
                    ]
                    for rk in range(2)
                ]
                return agin, agout, xr

            agin1, agout1, xr1 = make_ag(0, KC_OUT[0], dt.float8e4)
            agin2, agout2, xr2 = make_ag(1, KC_OUT[1], dt.bfloat16)

            def ship_half(l, half, agin, agout, xr, xc, kco):
                """Ship a half-layer's 2 chunks: AllGather + readbacks."""
                for cc in range(2):
                    c = 2 * half + cc
                    nc.sync.dma_start(
                        agin[half][:, cc * 4 * kco : (cc + 1) * 4 * kco],
                        xc[c][:, :, :],
                    )
                nc.gpsimd.collective_compute(
                    "AllGather", mybir.AluOpType.bypass, replica_groups=RG,
                    ins=[agin[half].opt()], outs=[agout[half].opt()],
                )
                for rk in range(2):
                    for cc in range(2):
                        c = 2 * half + cc
                        nc.scalar.dma_start(
                            xr[rk][c][:, :, :],
                            agout[half][
                                rk * 128 : (rk + 1) * 128,
                                cc * 4 * kco : (cc + 1) * 4 * kco,
                            ],
                        )

            def mlp_chunk(l, c, agg_list, wa_list, xc_tile, mblk):
                """MLP + epilogue for output chunk c of layer l.

                agg_list: agg PSUM APs whose bf16 DVE copies accumulate into
                h1 against the matching wa_list stationary weights.
                """
                kco = KC_OUT[l]
                h1_ps = ps_mlp.tile([128, 512], dt.float32, tag="h1", name=f"h1_{l}_{c}")
                asbs = []
                for gi, agg_ps in enumerate(agg_list):
                    kp = agg_ps.shape[0]
                    agg_sb = wpool.tile(
                        [kp, 512], dt.bfloat16, tag=f"aggsb{gi}", name=f"aggsb_{l}_{c}_{gi}"
                    )
                    nc.vector.tensor_copy(agg_sb[:], agg_ps[:])
                    asbs.append(agg_sb)
                for gi, (agg_sb, wa) in enumerate(zip(asbs, wa_list)):
                    nc.tensor.matmul(
                        h1_ps[:], wa[:], agg_sb[:],
                        start=(gi == 0), stop=(gi == len(asbs) - 1),
                    )
                h1_sb = wpool.tile([128, 512], dt.bfloat16, tag="h1sb", name=f"h1sb_{l}_{c}")
                nc.scalar.activation(
                    h1_sb[:], h1_ps[:], mybir.ActivationFunctionType.Relu,
                    bias=ba_sb[l][:, 0:1],
                )
                xn_ps = ps_mlp.tile([128, 4 * kco], dt.float32, tag="xn", name=f"xn_{l}_{c}")
                # D-init: ones.T @ (D4/128) preloads the folded bias
                nc.tensor.matmul(xn_ps[:], ones_sb[:], d4v_sb[l][:], start=True, stop=False)
                for t2 in range(4):
                    nc.tensor.matmul(
                        xn_ps[:, t2 * kco : (t2 + 1) * kco],
                        h1_sb[:, t2 * 128 : (t2 + 1) * 128],
                        wb_sb[l][:],
                        start=False,
                        stop=(t2 == 3),
                    )
                # epilogue: out = (xn + D) * mask, cast on write
                nc.vector.tensor_mul(
                    xc_tile[:, :, :], xn_ps[:], mblk[:, c * 4 * kco : (c + 1) * 4 * kco]
                )

            xc1 = [
                xpool.tile([128, 4, KC_OUT[0]], dt.float8e4, tag=f"xc1_{c}", name=f"xc1_{c}")
                for c in range(4)
            ]
            xc2 = [
                xpool.tile([128, 4, KC_OUT[1]], dt.bfloat16, tag=f"xc2_{c}", name=f"xc2_{c}")
                for c in range(4)
            ]
            xout = [
                wpool.tile([128, 4, KC_OUT[2]], dt.float32, tag=f"xo{c}", name=f"xo_{c}")
                for c in range(4)
            ]

            # ================= Layer 0: fp8 DoubleRow aggregation ============
            for half in range(2):
                agg0 = [
                    ps_agg.tile([64, 512], dt.float32, tag=f"agg{cc}", name=f"agg0_{half}_{cc}")
                    for cc in range(2)
                ]
                for t in range(NPAIR):
                    lhsT = x0_sb[:, 2 * t : 2 * t + 2, :]
                    for cc in range(2):
                        c = 2 * half + cc
                        nc.tensor.matmul(
                            agg0[cc][:], lhsT,
                            adjp_sb[t][:, :, c * 512 : (c + 1) * 512],
                            start=(t == 0), stop=(t == NPAIR - 1), perf_mode=DR,
                        )
                    if half == 0 and 0 < t < 13:
                        # filler matmuls keep the PE HAM clock warm while the
                        # adjacency stream paces layer 0 (they execute during
                        # the DMA stalls; worst-case cost is small)
                        for _ in range(2):
                            nc.tensor.matmul(
                                wu_ps[:], wu_lhs[:], wu_rhs[:], start=True, stop=True
                            )
                for cc in range(2):
                    c = 2 * half + cc
                    mlp_chunk(0, c, [agg0[cc]], [wa0_sb], xc1[c], mblk01_sb)
                ship_half(0, half, agin1, agout1, xr1, xc1, KC_OUT[0])

            # ================= Layer 1: fp8 DoubleRow ========================
            pair_order = [(rk, c, u) for c in range(4) for rk in range(2) for u in range(2)]
            for half in range(2):
                agg1 = [
                    [
                        ps_agg.tile(
                            [64, 512], dt.float32, tag=f"agg{2 * cc + g}",
                            name=f"agg1_{half}_{cc}_{g}",
                        )
                        for g in range(2)
                    ]
                    for cc in range(2)
                ]
                for pi, (rk, c, u) in enumerate(pair_order):
                    t = rk * 8 + c * 2 + u  # adjacency pair index
                    for g in range(2):
                        lhsT = xr1[rk][c][:, 2 * u : 2 * u + 2, 64 * g : 64 * g + 64]
                        for cc in range(2):
                            c_out = 2 * half + cc
                            nc.tensor.matmul(
                                agg1[cc][g][:], lhsT,
                                adjp_sb[t][:, :, c_out * 512 : (c_out + 1) * 512],
                                start=(pi == 0), stop=(pi == NPAIR - 1), perf_mode=DR,
                            )
                for cc in range(2):
                    c_out = 2 * half + cc
                    mlp_chunk(
                        1, c_out, [agg1[cc][0], agg1[cc][1]], [wa1a_sb, wa1b_sb],
                        xc2[c_out], mblk01_sb,
                    )
                ship_half(1, half, agin2, agout2, xr2, xc2, KC_OUT[1])

            # ================= Layer 2: bf16 x against fp8 adjacency =========
            # quarter-split, software-pipelined: emit chunk c+1's aggs before
            # chunk c's MLP so the PE never stalls on the psum->sbuf copy.
            tile_order = [(rk, c, u) for c in range(4) for rk in range(2) for u in range(4)]
            agg2 = [None] * 4

            def l2_aggs(c_out):
                agg2[c_out] = ps_agg.tile(
                    [128, 512], dt.float32, tag=f"agg{c_out % 3}", name=f"agg2_{c_out}"
                )
                for ti, (rk, c, u) in enumerate(tile_order):
                    t, j = divmod(rk * 16 + c * 4 + u, 2)  # adjacency pair, slot
                    lhsT = xr2[rk][c][:, u : u + 1, :]
                    nc.tensor.matmul(
                        agg2[c_out][:], lhsT,
                        adjp_sb[t][:, j : j + 1, c_out * 512 : (c_out + 1) * 512],
                        start=(ti == 0), stop=(ti == len(tile_order) - 1),
                    )

            def l2_mlp(c_out):
                mlp_chunk(2, c_out, [agg2[c_out]], [wa2_sb], xout[c_out], mblk2_sb)
                nc.sync.dma_start(
                    out_d[:, c_out * 4 * KC_OUT[2] : (c_out + 1) * 4 * KC_OUT[2]],
                    xout[c_out][:, :, :],
                )

            l2_aggs(0)
            l2_aggs(1)
            l2_mlp(0)
            l2_aggs(2)
            l2_mlp(1)
            l2_aggs(3)
            l2_mlp(2)
            l2_mlp(3)

    n_split = _legalize_sync_waits(nc)
    print(f"kernel: legalized {n_split} multi-wait instructions", file=sys.stderr)
    return nc


def get_program():
    if "nc" not in _PROGRAM_CACHE:
        _PROGRAM_CACHE["nc"] = _build_program()
    return _PROGRAM_CACHE["nc"]


def prepare_in_maps(inputs):
    """Host-side prep: fold BN into weights, transpose+slice adjacency, quantize x."""
    f32 = np.float32
    x = np.asarray(inputs["x"], f32)
    adj = np.asarray(inputs["adj"], f32)
    mask = np.asarray(inputs["mask"]).astype(bool)

    # folded per-layer constants (shared by all cores)
    const = {}
    for l in range(3):
        Wa = np.asarray(inputs[f"Wa{l}"], f32)
        ba = np.asarray(inputs[f"ba{l}"], f32)
        Wb = np.asarray(inputs[f"Wb{l}"], f32)
        bb = np.asarray(inputs[f"bb{l}"], f32)
        s1 = np.asarray(inputs[f"bng{l}"], f32) / np.sqrt(
            np.asarray(inputs[f"bnv{l}"], f32) + BN_EPS
        )
        c1 = np.asarray(inputs[f"bnb{l}"], f32) - np.asarray(inputs[f"bnm{l}"], f32) * s1
        Wb1 = s1[:, None] * Wb
        bb1 = bb + c1 @ Wb
        if l < 2:
            s2 = np.asarray(inputs[f"og{l}"], f32) / np.sqrt(
                np.asarray(inputs[f"ov{l}"], f32) + BN_EPS
            )
            c2 = np.asarray(inputs[f"ob{l}"], f32) - np.asarray(inputs[f"om{l}"], f32) * s2
            Wb2 = (Wb1 * s2[None, :]).astype(f32)
            d = (bb1 * s2 + c2).astype(f32)
        else:
            Wb2 = Wb1.astype(f32)
            d = bb1.astype(f32)
        ci, co = Wa.shape[0], Wb2.shape[1]
        waBD = np.zeros((2 * ci, 2 * H), f32)
        wbBD = np.zeros((2 * H, 2 * co), f32)
        for k in range(2):
            waBD[k * ci : (k + 1) * ci, k * H : (k + 1) * H] = Wa
            wbBD[k * H : (k + 1) * H, k * co : (k + 1) * co] = Wb2
        if l == 0:
            const["wa0"] = waBD.astype(BF16)
        elif l == 1:
            const["wa1a"] = np.ascontiguousarray(waBD[0:64, :]).astype(BF16)
            const["wa1b"] = np.ascontiguousarray(waBD[64:128, :]).astype(BF16)
        else:
            const["wa2"] = waBD.astype(BF16)
        const[f"wb{l}"] = wbBD.astype(BF16)
        const[f"ba{l}"] = np.concatenate([ba, ba]).reshape(128, 1).astype(f32)
        d2 = np.concatenate([d, d]) / 128.0
        const[f"d4v{l}"] = np.broadcast_to(
            np.tile(d2, 4)[None, :], (128, 4 * 2 * co)
        ).astype(BF16).copy()

    in_maps = []
    for core in range(N_CORES):
        b, half = divmod(core, 2)
        r0 = half * HALF
        # adjT[i, m] = adj[b][r0+m, i] + I -> pair-interleaved p-major layout
        adjT = np.ascontiguousarray(adj[b][r0 : r0 + HALF, :].T)
        adjT[np.arange(HALF) + r0, np.arange(HALF)] += 1.0
        adjp = (
            adjT.reshape(NPAIR, 2, 128, HALF)
            .transpose(0, 2, 1, 3)
            .reshape(NPAIR * 128, 2 * HALF)
            .astype(FP8)
        )
        xb = x[b].reshape(N, KC_IN[0]).astype(FP8)  # |x| <= ~5.1, no clip needed
        x0q = xb.reshape(32, 128, KC_IN[0]).transpose(1, 0, 2).reshape(128, 32 * KC_IN[0])
        mhalf = mask[b][r0 : r0 + HALF].astype(f32)
        # mask blocks [p, chunk*4*kco + u*kco + ch] = mask[chunk*512+u*128+p]
        mcols = mhalf.reshape(16, 128).T  # [128, 16]
        m = dict(const)
        m["mblk01"] = np.ascontiguousarray(
            np.repeat(mcols, KC_OUT[0], axis=1)
        ).astype(BF16)
        m["mblk2"] = np.ascontiguousarray(
            np.repeat(mcols, KC_OUT[2], axis=1)
        ).astype(BF16)
        m["adjp"] = adjp
        m["x0q"] = np.ascontiguousarray(x0q)
        in_maps.append(m)
    return in_maps


def gather_output(res):
    """Unshuffle per-core p-major outputs into the full (B,N,K,C_OUT) tensor."""
    out = np.zeros((B, N, K, C_OUT), np.float32)
    for core in range(N_CORES):
        b, half = divmod(core, 2)
        r0 = half * HALF
        o = res.results[core]["out"].reshape(128, 16, KC_OUT[2])
        o = o.transpose(1, 0, 2).reshape(HALF, K, C_OUT)
        out[b, r0 : r0 + HALF] = o
    return out


def run(in_maps, trace=False, **kw):
    nc = get_program()
    return run_bass_kernel_spmd(nc, in_maps, list(range(N_CORES)), trace=trace, **kw)


def kernel(**inputs) -> np.ndarray:
    in_maps = prepare_in_maps(inputs)
    res = run(in_maps)
    return gather_output(res)
